# revision 25
# baseline (speedup 1.0000x reference)
"""Causal self-attention (B=4, S=2048, D=1024, H=16, rope) on 8 trn2 cores.

Sharding: batch x head-half. Core c handles batch b=c//2 and heads
hh*8..hh*8+7 where hh=c%2. Each core computes its 8 heads' attention over its
batch and a partial output projection; the host sums the two partials per
batch.

v3 changes (vs the 458-500us v2 baseline):
- All matmul inputs are bf16 (tolerance is 2e-2; bf16 lands ~5e-3). PE column
  rate is dtype-independent (1 col/cycle @2.4GHz warm), so the win is not PE
  rate: it kills the f32r weight-rounding CAST stage (51us of gpsimd that
  serialized ph1), halves DMA-in bytes, and halves SBUF traffic.
- x arrives host-pre-transposed (xt [D,S]), removing 128 PE transposes and 16
  ACT copies, and the x staging pipeline that stalled ph1.
- Projections ordered for stationary reuse: (w, oc) outer, ic, then 4 s-chunks
  sharing one LDWEIGHTS.
- Attention processes ONE 512-q chunk at a time (v2 did two): score psum pool
  gets real double buffering (2x[128,1024] = 4 banks) + pv-out 4 banks = 8,
  so the PE can run a chunk ahead of the ACT exp instead of ping-ponging.
  v2's ping-pong idled both engines and HAM-cold-clocked the PE (~174us of
  the span at K=4/8).
- Output projection DMAs y straight from PSUM (no ACT copy).
- exp() on ACT is the attention pace-setter: 160 calls x ~967ns = 155us.
  PE attention work is ~150us; they pipeline now.
"""

import numpy as np
import ml_dtypes

B, S, D, H, DK = 4, 2048, 1024, 16, 64
THETA = 10000.0
N_CORES = 8
HPC = H // 2          # heads per core
OC = 4                # head-pairs per core (128-feature blocks)
SC4 = 4               # 512-wide s chunks
NKT = S // 128        # k tiles
VROW = 584            # per-j v-slab row: 8*65 + 64 pad so head 7's 128-col
                      # stationary slice stays in bounds

BF16 = ml_dtypes.bfloat16
_prog_cache = {}


def _apply_walrus_wait_workarounds():
    """This container's walrus rejects any TPB instruction with more than one
    sync wait. Patch the Tile kernel-tail drain to emit a chain of single-wait
    drains, and provide a post-pass that hoists excess waits onto NoOps."""
    import concourse.mybir as mybir
    import concourse.tile as tile_mod
    from concourse.vector_clock import ScopedClock

    def _drain_and_barrier(self, tick_clock, wait_clock):
        nc = self.nc
        drain_inst = nc.sync.drain()
        wait_clock.add_sem_waits(
            drain_inst.ins, ScopedClock({None: tick_clock.global_clock}))
        waits = list(drain_inst.ins.sync_info.on_wait)
        if len(waits) > 1:
            si = drain_inst.ins.sync_info
            si.on_wait = waits[:1]
            drain_inst.ins.sync_info = si
            for i in range(1, len(waits)):
                d2 = nc.sync.drain()
                d2.ins.sync_info = mybir.SyncInfo(
                    on_wait=waits[i:i + 1], on_update=[])
        nc.all_engine_barrier()
        popped = nc._tile_sem_poison_stack.pop()
        assert popped is self._sem_poison
        nc.clear_and_free_semaphores(list(self.sems.allocated().values()))
        nc.all_engine_barrier()

    tile_mod.TileContext._drain_and_barrier = _drain_and_barrier


def _split_waits(nc):
    import concourse.mybir as mybir
    engines = {mybir.EngineType.PE, mybir.EngineType.DVE, mybir.EngineType.SP,
               mybir.EngineType.Activation, mybir.EngineType.Pool}
    for f in nc.m.functions:
        for bb in f.blocks:
            out = []
            changed = False
            for ins in bb.instructions:
                si = ins.sync_info
                if si is not None and len(si.on_wait) > 1 and ins.engine in engines:
                    waits = list(si.on_wait)
                    for i in range(len(waits) - 1):
                        out.append(mybir.InstNoOp(
                            name=f"{ins.name}-waitsplit-{i}",
                            sync_info=mybir.SyncInfo(
                                on_wait=waits[i:i + 1], on_update=[]),
                            bass_nofuse=True, engine=ins.engine))
                    ins.sync_info = mybir.SyncInfo(
                        on_wait=waits[-1:], on_update=list(si.on_update))
                    changed = True
                out.append(ins)
            if changed:
                bb.instructions = out


def _build_program():
    _apply_walrus_wait_workarounds()
    import concourse.bass as bass
    import concourse.mybir as mybir
    import concourse.tile as tile
    from concourse.masks import make_identity
    from contextlib import ExitStack

    F32 = mybir.dt.float32
    F32R = mybir.dt.float32r
    BF = mybir.dt.bfloat16
    AF = mybir.ActivationFunctionType

    nc = bass.Bass()
    xt = nc.declare_dram_parameter("xt", [D, S], BF, isOutput=False)
    wqt = nc.declare_dram_parameter("wqt", [D, 512], BF, isOutput=False)
    wkt = nc.declare_dram_parameter("wkt", [D, 512], BF, isOutput=False)
    wvt = nc.declare_dram_parameter("wvt", [D, 512], BF, isOutput=False)
    wot = nc.declare_dram_parameter("wot", [512, D], F32, isOutput=False)
    cost = nc.declare_dram_parameter("cost", [128, S], F32, isOutput=False)
    sint2 = nc.declare_dram_parameter("sint2", [128, S], F32, isOutput=False)
    esel = nc.declare_dram_parameter("esel", [128, 16, 128], F32, isOutput=False)
    y = nc.declare_dram_parameter("y", [S, D], F32, isOutput=True)

    with tile.TileContext(nc) as tc, ExitStack() as ctx:
        singles = ctx.enter_context(tc.tile_pool(name="singles", bufs=1))

        # persistent slabs
        qslab = singles.tile([128, OC, S], BF, tag="qslab")
        kslab = singles.tile([128, OC, S], BF, tag="kslab")
        vslab = singles.tile([128, NKT, VROW], BF, tag="vslab")
        aout = singles.tile([128, OC, S], F32R, tag="aout")   # attn out (f32r)
        ones_col = singles.tile([128, NKT, 1], BF, tag="ones_col")
        nc.vector.memset(ones_col, 1.0)
        for h in range(HPC):
            nc.vector.tensor_copy(
                vslab[:, :, 65 * h + 64:65 * h + 65], ones_col)

        # causal-mask helpers (bf16): trimask[m, n] = 1 where n < m; negid
        # = -1e30 * I. Folded into the QK psum so exp gives exact zeros.
        trimask = singles.tile([128, 128], BF, tag="trimask")
        negid = singles.tile([128, 128], BF, tag="negid")
        # norm-phase selector + sums
        wor = singles.tile([128, 4, D], F32R, tag="wor")
        esl = singles.tile([128, 16, 128], F32R, tag="esl")
        sums = singles.tile([128, 512], F32, tag="sums")
        recips4 = singles.tile([128, 512], F32R, tag="recips4")
        rf = singles.tile([128, 512], F32, tag="rf")
        nc.vector.memset(rf, 0.0)
        nc.vector.tensor_copy(recips4, rf)

        # PE warmup: ~10us of dummy matmuls so HAM unthrottles while the
        # initial DMAs land; plus a tiny exp to pull the ACT table forward.
        with tc.tile_pool(name="warm", bufs=1) as warm, \
             tc.tile_pool(name="warmp", bufs=1, space="PSUM") as warmp:
            ident = warm.tile([128, 128], F32, tag="ident")
            make_identity(nc, ident)
            tm_stage = warm.tile([128, 128], F32, tag="tm_stage")
            nc.vector.memset(tm_stage, 1.0)
            nc.gpsimd.affine_select(
                out=tm_stage, in_=tm_stage,
                compare_op=mybir.AluOpType.is_ge,
                fill=0.0, base=-1, pattern=[[-1, 128]], channel_multiplier=1)
            nc.vector.tensor_copy(trimask, tm_stage)
            nid_stage = warm.tile([128, 128], F32, tag="nid_stage")
            nc.vector.tensor_scalar_mul(nid_stage, ident, -1.0e30)
            nc.vector.tensor_copy(negid, nid_stage)

            zpad = warm.tile([128, NKT, VROW - 520], BF, tag="zpad")
            nc.vector.memset(zpad, 0.0)
            nc.vector.tensor_copy(vslab[:, :, 520:VROW], zpad)

            wt = warm.tile([128, 512], F32, tag="warm")
            wtf = warm.tile([128, 16], F32, tag="warmf")
            nc.vector.memset(wt, 0.0)
            nc.vector.memset(wtf, 0.0)
            nc.scalar.activation(out=wtf, in_=wtf, func=AF.Exp, scale=1.0)
            wp = warmp.tile([128, 512], F32, tag="warmps")
            for i in range(26):
                nc.tensor.matmul(wp, lhsT=wt[:, 0:128], rhs=wt,
                                 start=True, stop=True)

        # ---------------- phase 1: project q/k/v, rope ----------------------
        with tc.tile_pool(name="wpool", bufs=1) as wpool, \
             tc.tile_pool(name="xtp", bufs=1) as xtp, \
             tc.tile_pool(name="ropetmp", bufs=5) as ropetmp, \
             tc.tile_pool(name="cspool", bufs=1) as cspool, \
             tc.tile_pool(name="psp", bufs=8, space="PSUM") as psp:

            # weights + xT: straight bf16 DMAs, no rounding stage needed.
            # Emission order = consumption order: wq, xT (first matmul), then
            # rope tables (first rope), then wk, wv.
            wr = {}
            for name, src in (("q", wqt), ("k", wkt), ("v", wvt)):
                wr[name] = wpool.tile([128, 8, 512], BF, tag=f"w{name}",
                                      name=f"w{name}")
            wq_r = wqt.rearrange("(ic p) o -> p ic o", p=128)
            for ic in range(8):
                nc.sync.dma_start(out=wr["q"][:, ic, :], in_=wq_r[:, ic, :])

            xts = xtp.tile([128, 8, S], BF, tag="xts")
            xt_r = xt.rearrange("(ic p) s -> p ic s", p=128)
            for ic in range(8):
                nc.sync.dma_start(out=xts[:, ic, :], in_=xt_r[:, ic, :])

            cosc = cspool.tile([128, S], F32, tag="cosc")
            nc.sync.dma_start(out=cosc, in_=cost[:])
            sinc = cspool.tile([128, S], F32, tag="sinc")
            nc.sync.dma_start(out=sinc, in_=sint2[:])

            for name, src in (("k", wkt), ("v", wvt)):
                src_r = src.rearrange("(ic p) o -> p ic o", p=128)
                for ic in range(8):
                    nc.sync.dma_start(out=wr[name][:, ic, :],
                                      in_=src_r[:, ic, :])

            # q/k projections with rope; stationary (w chunk) reused across
            # the 4 s-chunks of each (w, oc, ic) group.
            swap = _pair_swap_mask()
            rope_n = 0
            for wname, slab in (("q", qslab), ("k", kslab)):
                for oc in range(OC):
                    pps = [psp.tile([128, 512], F32, tag="pp",
                                    name=f"pp{wname}{oc}_{s4}")
                           for s4 in range(SC4)]
                    for ic in range(8):
                        for s4 in range(SC4):
                            nc.tensor.matmul(
                                pps[s4],
                                lhsT=wr[wname][:, ic, oc * 128:(oc + 1) * 128],
                                rhs=xts[:, ic, s4 * 512:(s4 + 1) * 512],
                                start=(ic == 0), stop=(ic == 7))
                    for s4 in range(SC4):
                        ssl = slice(s4 * 512, (s4 + 1) * 512)
                        pp = pps[s4]
                        tsh = ropetmp.tile([128, 512], F32, tag="tsh")
                        nc.vector.stream_shuffle(tsh, pp, swap)
                        tcs = ropetmp.tile([128, 512], F32, tag="tcs")
                        nc.vector.tensor_mul(tcs, pp, cosc[:, ssl])
                        nc.gpsimd.tensor_mul(tsh, tsh, sinc[:, ssl])
                        # alternate the final add between DVE and gpsimd so
                        # neither engine paces the psum drain
                        if rope_n % 2 == 0:
                            nc.vector.tensor_add(slab[:, oc, ssl], tcs, tsh)
                        else:
                            nc.gpsimd.tensor_add(slab[:, oc, ssl], tcs, tsh)
                        rope_n += 1

            # v projection (natural [s, o] layout), stationary = x s-tile
            for st in range(16):
                pv = psp.tile([128, 512], F32, tag="pp", name=f"pv{st}")
                for ic in range(8):
                    nc.tensor.matmul(
                        pv, lhsT=xts[:, ic, st * 128:(st + 1) * 128],
                        rhs=wr["v"][:, ic, :],
                        start=(ic == 0), stop=(ic == 7))
                nc.scalar.copy(
                    out=vslab[:, st, 0:520].rearrange(
                        "p (h d) -> p h d", d=65)[:, :, 0:64],
                    in_=pv.rearrange("p (h dk) -> p h dk", h=HPC))

        # ---------------- phase 2: attention --------------------------------
        with tc.tile_pool(name="wostage", bufs=2) as wostage:
            # prefetch output-projection weights + selector during attention
            wot_r = wot.rearrange("(ic p) o -> p ic o", p=128)
            for ic in range(4):
                wst = wostage.tile([128, D], F32, tag="wost", name=f"wst{ic}")
                nc.sync.dma_start(out=wst, in_=wot_r[:, ic, :])
                nc.gpsimd.tensor_copy(wor[:, ic, :], wst)
            esl_st = wostage.tile([128, 16, 128], F32, tag="esl_st")
            nc.sync.dma_start(out=esl_st, in_=esel[:])
            nc.gpsimd.tensor_copy(esl, esl_st)

            with tc.tile_pool(name="ptpool", bufs=3) as ptpool, \
                 tc.tile_pool(name="stmp", bufs=4) as stmpp, \
                 tc.tile_pool(name="pss", bufs=2, space="PSUM") as pss, \
                 tc.tile_pool(name="pso", bufs=3, space="PSUM") as pso, \
                 tc.tile_pool(name="psn", bufs=1, space="PSUM") as psn:

                slots = [(hp, c, j)
                         for hp in range(OC)
                         for c in range(4)
                         for j in range(4 * c + 4)]
                ps_tiles = {}

                def emit_qk(slot):
                    hp, c, j = slot
                    d = max(0, j * 128 - c * 512)
                    diag = (j // 4 == c)
                    ps = ps_tiles[slot] = pss.tile([128, 1024], F32,
                                                   tag="ps", name="ps")
                    for bi in range(2):
                        r0 = bi * 64
                        nc.tensor.matmul(
                            ps[:, bi * 512 + d:(bi + 1) * 512],
                            lhsT=kslab[r0:r0 + 64, hp,
                                       j * 128:(j + 1) * 128],
                            rhs=qslab[r0:r0 + 64, hp,
                                      c * 512 + d:(c + 1) * 512],
                            start=True, stop=not diag)
                    if diag:
                        for bi in range(2):
                            nc.tensor.matmul(
                                ps[:, bi * 512 + d:bi * 512 + d + 128],
                                lhsT=negid, rhs=trimask,
                                start=False, stop=True,
                                skip_group_check=True)

                def emit_norm_recip(hp):
                    # recip of this head-pair's 8 denominator rows + f32r
                    # rounding copy, in 4 column pieces so writeback copies
                    # can interleave on the DVE
                    r8 = slice(32 * hp, 32 * hp + 8)
                    for piece in range(4):
                        fs = slice(piece * 128, piece * 128 + 128)
                        nc.vector.reciprocal(rf[r8, fs], sums[r8, fs])
                    nc.vector.tensor_copy(recips4[r8, :], rf[r8, :])

                def emit_norm_mm(hp):
                    # broadcast via selector matmuls + scale aout. Emitted two
                    # chunks after the recip so the pb matmuls never sit in
                    # the PE FIFO waiting on the DVE chain (the PE semaphore
                    # is a global counter - a stalled matmul stalls every
                    # later exp too).
                    for c in range(4):
                        pb = psn.tile([128, 512], F32, tag="pb", name="pb")
                        nc.tensor.matmul(pb, lhsT=esl[:, hp * 4 + c, :],
                                         rhs=recips4,
                                         start=True, stop=True)
                        qsl = slice(c * 512, (c + 1) * 512)
                        nc.vector.tensor_mul(
                            aout[:, hp, qsl], aout[:, hp, qsl], pb)

                pos = {}
                norm_queue = []
                emit_qk(slots[0])
                for i, slot in enumerate(slots):
                    if i + 1 < len(slots):
                        emit_qk(slots[i + 1])
                    hp, c, j = slot
                    d = max(0, j * 128 - c * 512)
                    ps = ps_tiles.pop(slot)
                    pt = ptpool.tile([128, 1024], BF, tag="pt")
                    nc.scalar.activation(
                        out=pt.rearrange(
                            "p (b c) -> p b c", b=2)[:, :, d:512],
                        in_=ps.rearrange(
                            "p (b c) -> p b c", b=2)[:, :, d:512],
                        func=AF.Exp, scale=0.125)
                    if j == 0:
                        pos[hp] = [pso.tile([128, 512], F32, tag="po",
                                            name=f"po{hp}_{c}_{bi}")
                                   for bi in range(2)]
                    for bi in range(2):
                        h = 2 * hp + bi
                        nc.tensor.matmul(
                            pos[hp][bi][:, d:512],
                            lhsT=vslab[:, j, 65 * h:65 * h + 128],
                            rhs=pt[:, bi * 512 + d:(bi + 1) * 512],
                            start=(j == 0), stop=(j == 4 * c + 3))
                    if j == 4 * c + 3:
                        qsl = slice(c * 512, (c + 1) * 512)
                        for bi in range(2):
                            po = pos[hp][bi]
                            r0 = bi * 64
                            nc.vector.tensor_copy(
                                aout[r0:r0 + 64, hp, qsl], po[0:64, :])
                            stmp = stmpp.tile([1, 512], F32, tag="stmp")
                            nc.vector.tensor_copy(stmp, po[64:65, :])
                            hc = 32 * hp + 4 * bi + c
                            nc.sync.dma_start(
                                out=sums[hc:hc + 1, :], in_=stmp)
                        # norm stages for a finished head-pair trail by one
                        # (recip) and three (pb matmuls) writebacks, so the
                        # pb matmuls never enter the PE FIFO before the DVE
                        # recip chain has drained
                        nxt = []
                        for nhp, age in norm_queue:
                            age += 1
                            if age == 1:
                                emit_norm_recip(nhp)
                            if age == 3:
                                emit_norm_mm(nhp)
                            else:
                                nxt.append((nhp, age))
                        norm_queue = nxt
                        if c == 3:
                            norm_queue.append((hp, 0))
                # tail: keep the PE busy while hp3's recip chain runs, else
                # HAM re-throttles and ph4 starts at half clock
                for kw in range(5):
                    junk = psn.tile([128, 512], F32, tag="pb", name="kw")
                    nc.tensor.matmul(junk, lhsT=negid,
                                     rhs=qslab[:, 0, 0:512],
                                     start=True, stop=True)
                for nhp, age in norm_queue:
                    if age < 1:
                        emit_norm_recip(nhp)
                    emit_norm_mm(nhp)

            # ------------- phase 4: output projection --------------------
            with tc.tile_pool(name="ysb", bufs=3) as ysb, \
                 tc.tile_pool(name="psy", bufs=4, space="PSUM") as psy:
                for qs in range(16):
                    pys = [psy.tile([128, 512], F32, tag="py",
                                    name=f"py{qs}_{oh}") for oh in range(2)]
                    for ic in range(4):
                        for oh in range(2):
                            nc.tensor.matmul(
                                pys[oh],
                                lhsT=aout[:, ic, qs * 128:(qs + 1) * 128],
                                rhs=wor[:, ic, oh * 512:(oh + 1) * 512],
                                start=(ic == 0), stop=(ic == 3))
                    yt = ysb.tile([128, D], F32, tag="yt")
                    for oh in range(2):
                        nc.scalar.copy(
                            out=yt[:, oh * 512:(oh + 1) * 512], in_=pys[oh])
                    nc.sync.dma_start(out=y[qs * 128:(qs + 1) * 128, :], in_=yt)

    _split_waits(nc)
    return nc


def _pair_swap_mask():
    mask = []
    for j in range(16):
        mask += [2 * j + 1, 2 * j]
    return mask


def _host_inputs(x, wq, wk, wv, wo, token_positions):
    pos = np.asarray(token_positions).astype(np.float64)
    ex = np.arange(0, DK, 2, dtype=np.float64) / DK
    freq = 1.0 / (THETA ** ex)
    f = pos[:, None] * freq[None, :]                       # [S, DK/2]
    cos = np.repeat(np.cos(f), 2, axis=1).astype(np.float32)   # [S, DK]
    sin = np.repeat(np.sin(f), 2, axis=1).astype(np.float32)
    cosT = np.ascontiguousarray(cos.T)                     # [DK, S]
    sinT = np.ascontiguousarray(sin.T)
    sgn = np.where(np.arange(DK) % 2 == 0, -1.0, 1.0).astype(np.float32)
    sinT2 = sinT * sgn[:, None]
    cost = np.tile(cosT, (2, 1))                           # [128, S]
    sint2 = np.tile(sinT2, (2, 1))

    # selector matrices for the denominator-broadcast matmul: within a
    # head-pair's 32-row sums block, row (m>=64)*4 + c holds the denominators
    # for output partition m, q-chunk c. Replicated x4 on the contraction dim
    # (recips4 rows) with 0.25 scale so the matmul uses all 128 PE rows.
    esel4 = np.zeros((128, 16, 128), np.float32)
    for hp in range(4):
        for c in range(4):
            for bi in range(2):
                esel4[32 * hp + 4 * bi + c, hp * 4 + c,
                      bi * 64:(bi + 1) * 64] = 1.0

    wqT = np.ascontiguousarray(wq.T)
    wkT = np.ascontiguousarray(wk.T)
    wvT = np.ascontiguousarray(wv.T)
    woT = np.ascontiguousarray(wo.T)

    xts = [np.ascontiguousarray(x[b].T).astype(BF16) for b in range(B)]

    in_maps = []
    for core in range(N_CORES):
        b, hh = core // 2, core % 2
        osl = slice(hh * 512, (hh + 1) * 512)
        in_maps.append({
            "xt": xts[b],
            "wqt": np.ascontiguousarray(wqT[:, osl]).astype(BF16),
            "wkt": np.ascontiguousarray(wkT[:, osl]).astype(BF16),
            "wvt": np.ascontiguousarray(wvT[:, osl]).astype(BF16),
            "wot": np.ascontiguousarray(woT[osl, :]),
            "cost": cost,
            "sint2": sint2,
            "esel": esel4,
        })
    return in_maps


def run_sharded(x, wq, wk, wv, wo, token_positions, trace=False):
    from concourse.bass_utils import run_bass_kernel_spmd
    if "nc" not in _prog_cache:
        _prog_cache["nc"] = _build_program()
    nc = _prog_cache["nc"]
    in_maps = _host_inputs(x, wq, wk, wv, wo, token_positions)
    res = run_bass_kernel_spmd(nc, in_maps, list(range(N_CORES)), trace=trace)
    out = np.empty((B, S, D), np.float32)
    for b in range(B):
        out[b] = res.results[2 * b]["y"] + res.results[2 * b + 1]["y"]
    return out, res


def kernel(x, wq, wk, wv, wo, token_positions):
    x = np.asarray(x, dtype=np.float32)
    out, _ = run_sharded(
        x, np.asarray(wq, np.float32), np.asarray(wk, np.float32),
        np.asarray(wv, np.float32), np.asarray(wo, np.float32),
        np.asarray(token_positions))
    return out
